# revision 1
# baseline (speedup 1.0000x reference)
"""Trainium2 Bass kernel for nn_LlamaApproximatedAttention.

Math (per batch b, with hs = hidden_states[b] [S, H]):
    F_h = W_seq @ hs            # [R, H]   (contract s)
    F_s = hs @ W_hid.T          # [S, R]   (contract h)
    out = F_s @ F_h             # [S, H]   (contract r)

Sharding: 8 cores = (batch b = c//2, seq-half j = c%2). Each core receives the
full hs[b] with its own half's rows first (host-side roll), computes F_h fully,
and F_s / out only for its own half (dram rows 0..1023). Pure SPMD.

All PE inputs use float32r (full-rate fp32 matmul mode, ~1.5e-4 rel err).
grid_chw is unused by the math (it enumerates the full (s, h) grid).
"""

import numpy as np

import concourse.bass as bass  # noqa: F401  (engine namespaces hang off nc)
import concourse.mybir as mybir
import concourse.tile as tile
from concourse import bacc
from concourse.bass_utils import run_bass_kernel_spmd

B, S, H, R = 4, 2048, 2048, 64
N_CORES = 8
P = 128
T = S // P            # 16 s-tiles (also 16 h-tiles)
OWN_T = T // 2        # 8 own s-tiles per core
CHUNK = 512
NCH = H // CHUNK      # 4 h-chunks

f32r = mybir.dt.float32r
f32 = mybir.dt.float32


def build_nc(reps: int = 1, mode: str = "full"):
    """Build + bacc-compile the SPMD kernel. reps>1 wraps the body in a
    hardware For loop (used only for timing; output is idempotent).

    mode (timing diagnostics, progressively enables phases):
      "dma"  : input loads + output stores only
      "e1"   : + einsum1 matmuls
      "tr"   : + PE transposes
      "e2"   : + einsum2 matmuls/copies
      "full" : + einsum3 (the real kernel)
    """
    lvl = {"dma": 0, "e1": 1, "tr": 2, "e2": 3, "full": 4}[mode]
    nc = bacc.Bacc(
        "TRN2",
        target_bir_lowering=False,
        debug=False,
        enable_asserts=True,
        num_devices=N_CORES,
    )

    hs = nc.dram_tensor("hs", [S, H], f32r, kind="ExternalInput").ap()
    wst = nc.dram_tensor("wst", [P, T * R], f32r, kind="ExternalInput").ap()
    wht = nc.dram_tensor("wht", [P, T * R], f32r, kind="ExternalInput").ap()
    ident = nc.dram_tensor("ident", [P, P], f32r, kind="ExternalInput").ap()
    out = nc.dram_tensor("out", [S // 2, H], f32, kind="ExternalOutput").ap()

    # [8][128, 2, 2048] pair view: pair m covers s-tiles (2m, 2m+1)
    hs_p2 = hs.rearrange("(m w p) h -> m p w h", w=2, p=P)
    out_p2 = out.rearrange("(m w p) h -> m p w h", w=2, p=P)

    with tile.TileContext(nc) as tc:

        def body(_=None):
            with (
                tc.tile_pool(name="const", bufs=1) as cpool,
                tc.tile_pool(name="raw", bufs=6) as rpool,
                tc.tile_pool(name="hsT", bufs=3) as hpool,
                tc.tile_pool(name="fact", bufs=1) as fpool,
                tc.tile_pool(name="outsb", bufs=2) as opool,
            ):
                wst_t = cpool.tile([P, T * R], f32r, tag="wst")
                wht_t = cpool.tile([P, T * R], f32r, tag="wht")
                id_t = cpool.tile([P, P], f32r, tag="id")
                nc.gpsimd.dma_start(wst_t[:], wst)
                nc.gpsimd.dma_start(wht_t[:], wht)
                nc.gpsimd.dma_start(id_t[:], ident)

                fst_dup = fpool.tile([P, OWN_T * P], f32r, tag="fst")  # [128, 1024]
                fh_dup = fpool.tile([P, H], f32r, tag="fh")            # [128, 2048]
                if lvl < 4:
                    fake_out = cpool.tile([P, 2 * H], f32, tag="fake_out")
                    nc.gpsimd.memset(fake_out[:], 0.0)

                with tc.tile_pool(name="ps_fh", bufs=1, space="PSUM") as fh_ps_pool:
                    ps_fh = fh_ps_pool.tile([R, H], f32, tag="fh")     # [64, 2048]

                    with (
                        tc.tile_pool(name="ps_tr", bufs=2, space="PSUM") as tr_ps,
                        tc.tile_pool(name="ps_fs", bufs=2, space="PSUM") as fs_ps,
                    ):
                        raws = {}
                        for m in range(T // 2):
                            raw2 = rpool.tile([P, 2 * H], f32r, tag="raw")
                            nc.sync.dma_start(raw2[:], hs_p2[m])
                            raws[m] = raw2
                            # einsum1: accumulate partial F_h chunks
                            for w in range(2 if lvl >= 1 else 0):
                                t = 2 * m + w
                                for c in range(NCH):
                                    nc.tensor.matmul(
                                        ps_fh[:, c * CHUNK:(c + 1) * CHUNK],
                                        wst_t[:, t * R:(t + 1) * R],
                                        raw2[:, w * H + c * CHUNK:w * H + (c + 1) * CHUNK],
                                        start=(t == 0),
                                        stop=(t == T - 1),
                                    )
                            # einsum2 for own groups g=0 (pairs 0,1), g=1 (pairs 2,3)
                            if lvl >= 2 and m in (1, 3):
                                g = m // 2
                                ps_fs_g = None
                                if lvl >= 3:
                                    ps_fs_g = fs_ps.tile([R, CHUNK], f32, tag="fs")
                                for k in range(T):  # h-tiles
                                    ps_tr = tr_ps.tile([P, 4 * P], f32r, tag="tr")
                                    for q in range(4):  # s-tiles 4g+q
                                        src = raws[2 * g + q // 2][
                                            :, (q % 2) * H + k * P:(q % 2) * H + (k + 1) * P
                                        ]
                                        nc.tensor.matmul(
                                            ps_tr[:, q * P:(q + 1) * P],
                                            src,
                                            id_t[:],
                                            is_transpose=True,
                                            start=(q == 0),
                                            stop=(q == 3),
                                        )
                                    hsT = hpool.tile([P, 4 * P], f32r, tag="hsT")
                                    if k % 2 == 0:
                                        nc.vector.tensor_copy(hsT[:], ps_tr[:])
                                    else:
                                        nc.scalar.copy(hsT[:], ps_tr[:])
                                    if lvl >= 3:
                                        nc.tensor.matmul(
                                            ps_fs_g[:],
                                            wht_t[:, k * R:(k + 1) * R],
                                            hsT[:],
                                            start=(k == 0),
                                            stop=(k == T - 1),
                                        )
                                if lvl >= 3:
                                    # F_s.T chunk -> SBUF, duplicated to both halves
                                    nc.vector.tensor_copy(
                                        fst_dup[0:R, g * CHUNK:(g + 1) * CHUNK], ps_fs_g[:]
                                    )
                                    nc.scalar.copy(
                                        fst_dup[R:2 * R, g * CHUNK:(g + 1) * CHUNK], ps_fs_g[:]
                                    )

                    if lvl >= 4:
                        # F_h -> SBUF per chunk (fine-grained so einsum3 can
                        # start on chunk 0 while later chunks still copy),
                        # duplicated to both partition halves
                        for c in range(NCH):
                            sl = slice(c * CHUNK, (c + 1) * CHUNK)
                            if c % 2 == 0:
                                nc.vector.tensor_copy(fh_dup[0:R, sl], ps_fh[:, sl])
                                nc.scalar.copy(fh_dup[R:2 * R, sl], ps_fh[:, sl])
                            else:
                                nc.scalar.copy(fh_dup[0:R, sl], ps_fh[:, sl])
                                nc.vector.tensor_copy(fh_dup[R:2 * R, sl], ps_fh[:, sl])

                # einsum3 (K=64, row-packed pairs) + output stores
                with tc.tile_pool(name="ps_o", bufs=4, space="PSUM") as po:
                    for ip in range(OWN_T // 2):
                        if lvl < 4:
                            nc.sync.dma_start(out_p2[ip], fake_out[:])
                            continue
                        outsb = opool.tile([P, 2 * H], f32, tag="outsb")
                        for c in range(NCH):
                            for hf in range(2):
                                i = 2 * ip + hf
                                base = R * hf
                                ps_o = po.tile([P, CHUNK], f32, tag="o")
                                nc.tensor.matmul(
                                    ps_o[:],
                                    fst_dup[base:base + R, i * P:(i + 1) * P],
                                    fh_dup[base:base + R, c * CHUNK:(c + 1) * CHUNK],
                                    start=True,
                                    stop=True,
                                )
                                dst = outsb[:, hf * H + c * CHUNK:hf * H + (c + 1) * CHUNK]
                                if c % 2 == 0:
                                    nc.vector.tensor_copy(dst, ps_o[:])
                                else:
                                    nc.scalar.copy(dst, ps_o[:])
                        nc.sync.dma_start(out_p2[ip], outsb[:])

        if reps == 1:
            body()
        else:
            with tc.For_i(0, reps, 1):
                body()

    nc.compile()
    return nc


def _tile_weight(w_t: np.ndarray) -> np.ndarray:
    """[2048, 64] -> [128, 16*64] stack where tile t = cols [64t:64t+64]."""
    return np.ascontiguousarray(
        w_t.reshape(T, P, R).transpose(1, 0, 2).reshape(P, T * R)
    )


_NC_CACHE: dict = {}


def kernel(**inputs) -> np.ndarray:
    hs_all = np.asarray(inputs["hidden_states"], dtype=np.float32)
    w_seq = np.asarray(inputs["W_seq"], dtype=np.float32)
    w_hid = np.asarray(inputs["W_hid"], dtype=np.float32)

    if "nc" not in _NC_CACHE:
        _NC_CACHE["nc"] = build_nc(1)
    nc = _NC_CACHE["nc"]

    ident = np.eye(P, dtype=np.float32)
    wst_rolls = {}
    for j in range(2):
        wst_rolls[j] = _tile_weight(np.roll(w_seq.T, -(S // 2) * j, axis=0))
    wht_tiled = _tile_weight(np.ascontiguousarray(w_hid.T))

    in_maps = []
    for c in range(N_CORES):
        b, j = c // 2, c % 2
        hsb = hs_all[b]
        hs_c = hsb if j == 0 else np.ascontiguousarray(np.roll(hsb, -(S // 2), axis=0))
        in_maps.append(
            {"hs": hs_c, "wst": wst_rolls[j], "wht": wht_tiled, "ident": ident}
        )

    res = run_bass_kernel_spmd(nc, in_maps, core_ids=list(range(N_CORES)))

    out_full = np.empty((B, S, H), dtype=np.float32)
    for c in range(N_CORES):
        b, j = c // 2, c % 2
        out_full[b, j * (S // 2):(j + 1) * (S // 2), :] = res.results[c]["out"]
    return out_full



# revision 2
# speedup vs baseline: 1.2397x; 1.2397x over previous
"""Trainium2 Bass kernel for nn_LlamaApproximatedAttention.

Math (per batch b, with hs = hidden_states[b] [S, H]):
    F_h = W_seq @ hs            # [R, H]   (contract s)
    F_s = hs @ W_hid.T          # [S, R]   (contract h)
    out = F_s @ F_h             # [S, H]   (contract r)

Sharding: 8 cores = (batch b = c//2, seq-half j = c%2). Each core receives the
full hs[b] with its own half's rows first (host-side roll), computes F_h fully,
and F_s / out only for its own half (dram rows 0..1023). Pure SPMD.

All PE inputs are float16 (PSUM accumulation stays fp32): halves DMA traffic
(the bottleneck) vs f32, and PE transposes run 1.0 cycles/row vs 1.5 for f32r.
grid_chw is unused by the math (it enumerates the full (s, h) grid).
"""

import numpy as np

import concourse.bass as bass  # noqa: F401  (engine namespaces hang off nc)
import concourse.mybir as mybir
import concourse.tile as tile
from concourse import bacc
from concourse.bass_utils import run_bass_kernel_spmd

B, S, H, R = 4, 2048, 2048, 64
N_CORES = 8
P = 128
T = S // P            # 16 s-tiles (also 16 h-tiles)
OWN_T = T // 2        # 8 own s-tiles per core
CHUNK = 512
NCH = H // CHUNK      # 4 h-chunks

f16 = mybir.dt.float16
f32 = mybir.dt.float32


def build_nc(reps: int = 1, mode: str = "full"):
    """Build + bacc-compile the SPMD kernel. reps>1 wraps the body in a
    hardware For loop (used only for timing; output is idempotent).

    mode (timing diagnostics, progressively enables phases):
      "dma"  : input loads + output stores only
      "e1"   : + einsum1 matmuls
      "tr"   : + PE transposes
      "e2"   : + einsum2 matmuls/copies
      "full" : + einsum3 (the real kernel)
    """
    lvl = {"dma": 0, "e1": 1, "tr": 2, "e2": 3, "full": 4}[mode]
    nc = bacc.Bacc(
        "TRN2",
        target_bir_lowering=False,
        debug=False,
        enable_asserts=True,
        num_devices=N_CORES,
    )

    hs = nc.dram_tensor("hs", [S, H], f16, kind="ExternalInput").ap()
    wst = nc.dram_tensor("wst", [P, T * R], f16, kind="ExternalInput").ap()
    wht = nc.dram_tensor("wht", [P, T * R], f16, kind="ExternalInput").ap()
    ident = nc.dram_tensor("ident", [P, P], f16, kind="ExternalInput").ap()
    out = nc.dram_tensor("out", [S // 2, H], f16, kind="ExternalOutput").ap()

    # [8][128, 2, 2048] pair view: pair m covers s-tiles (2m, 2m+1)
    hs_p2 = hs.rearrange("(m w p) h -> m p w h", w=2, p=P)
    out_p2 = out.rearrange("(m w p) h -> m p w h", w=2, p=P)

    with tile.TileContext(nc) as tc:

        def body(_=None):
            with (
                tc.tile_pool(name="const", bufs=1) as cpool,
                tc.tile_pool(name="raw", bufs=6) as rpool,
                tc.tile_pool(name="hsT", bufs=3) as hpool,
                tc.tile_pool(name="fact", bufs=1) as fpool,
                tc.tile_pool(name="outsb", bufs=2) as opool,
            ):
                wst_t = cpool.tile([P, T * R], f16, tag="wst")
                wht_t = cpool.tile([P, T * R], f16, tag="wht")
                id_t = cpool.tile([P, P], f16, tag="id")
                nc.gpsimd.dma_start(wst_t[:], wst)
                nc.gpsimd.dma_start(wht_t[:], wht)
                nc.gpsimd.dma_start(id_t[:], ident)

                fst_dup = fpool.tile([P, OWN_T * P], f16, tag="fst")  # [128, 1024]
                fh_dup = fpool.tile([P, H], f16, tag="fh")            # [128, 2048]
                if lvl < 4:
                    fake_out = cpool.tile([P, 2 * H], f16, tag="fake_out")
                    nc.gpsimd.memset(fake_out[:], 0.0)

                with tc.tile_pool(name="ps_fh", bufs=1, space="PSUM") as fh_ps_pool:
                    ps_fh = fh_ps_pool.tile([R, H], f32, tag="fh")     # [64, 2048]

                    with (
                        tc.tile_pool(name="ps_tr", bufs=2, space="PSUM") as tr_ps,
                        tc.tile_pool(name="ps_fs", bufs=2, space="PSUM") as fs_ps,
                    ):
                        raws = {}
                        for m in range(T // 2):
                            raw2 = rpool.tile([P, 2 * H], f16, tag="raw")
                            nc.sync.dma_start(raw2[:], hs_p2[m])
                            raws[m] = raw2
                            # einsum1: accumulate partial F_h chunks
                            for w in range(2 if lvl >= 1 else 0):
                                t = 2 * m + w
                                for c in range(NCH):
                                    nc.tensor.matmul(
                                        ps_fh[:, c * CHUNK:(c + 1) * CHUNK],
                                        wst_t[:, t * R:(t + 1) * R],
                                        raw2[:, w * H + c * CHUNK:w * H + (c + 1) * CHUNK],
                                        start=(t == 0),
                                        stop=(t == T - 1),
                                    )
                            # einsum2 for own groups g=0 (pairs 0,1), g=1 (pairs 2,3)
                            if lvl >= 2 and m in (1, 3):
                                g = m // 2
                                ps_fs_g = None
                                if lvl >= 3:
                                    ps_fs_g = fs_ps.tile([R, CHUNK], f32, tag="fs")
                                for k in range(T):  # h-tiles
                                    ps_tr = tr_ps.tile([P, 4 * P], f16, tag="tr")
                                    for q in range(4):  # s-tiles 4g+q
                                        src = raws[2 * g + q // 2][
                                            :, (q % 2) * H + k * P:(q % 2) * H + (k + 1) * P
                                        ]
                                        nc.tensor.matmul(
                                            ps_tr[:, q * P:(q + 1) * P],
                                            src,
                                            id_t[:],
                                            is_transpose=True,
                                            start=(q == 0),
                                            stop=(q == 3),
                                        )
                                    hsT = hpool.tile([P, 4 * P], f16, tag="hsT")
                                    if k % 2 == 0:
                                        nc.vector.tensor_copy(hsT[:], ps_tr[:])
                                    else:
                                        nc.scalar.copy(hsT[:], ps_tr[:])
                                    if lvl >= 3:
                                        nc.tensor.matmul(
                                            ps_fs_g[:],
                                            wht_t[:, k * R:(k + 1) * R],
                                            hsT[:],
                                            start=(k == 0),
                                            stop=(k == T - 1),
                                        )
                                if lvl >= 3:
                                    # F_s.T chunk -> SBUF, duplicated to both halves
                                    nc.vector.tensor_copy(
                                        fst_dup[0:R, g * CHUNK:(g + 1) * CHUNK], ps_fs_g[:]
                                    )
                                    nc.scalar.copy(
                                        fst_dup[R:2 * R, g * CHUNK:(g + 1) * CHUNK], ps_fs_g[:]
                                    )

                    if lvl >= 4:
                        # F_h -> SBUF per chunk (fine-grained so einsum3 can
                        # start on chunk 0 while later chunks still copy),
                        # duplicated to both partition halves
                        for c in range(NCH):
                            sl = slice(c * CHUNK, (c + 1) * CHUNK)
                            if c % 2 == 0:
                                nc.vector.tensor_copy(fh_dup[0:R, sl], ps_fh[:, sl])
                                nc.scalar.copy(fh_dup[R:2 * R, sl], ps_fh[:, sl])
                            else:
                                nc.scalar.copy(fh_dup[0:R, sl], ps_fh[:, sl])
                                nc.vector.tensor_copy(fh_dup[R:2 * R, sl], ps_fh[:, sl])

                # einsum3 (K=64, row-packed pairs) + output stores
                with tc.tile_pool(name="ps_o", bufs=4, space="PSUM") as po:
                    for ip in range(OWN_T // 2):
                        if lvl < 4:
                            nc.sync.dma_start(out_p2[ip], fake_out[:])
                            continue
                        outsb = opool.tile([P, 2 * H], f16, tag="outsb")
                        for c in range(NCH):
                            for hf in range(2):
                                i = 2 * ip + hf
                                base = R * hf
                                ps_o = po.tile([P, CHUNK], f32, tag="o")
                                nc.tensor.matmul(
                                    ps_o[:],
                                    fst_dup[base:base + R, i * P:(i + 1) * P],
                                    fh_dup[base:base + R, c * CHUNK:(c + 1) * CHUNK],
                                    start=True,
                                    stop=True,
                                )
                                dst = outsb[:, hf * H + c * CHUNK:hf * H + (c + 1) * CHUNK]
                                if c % 2 == 0:
                                    nc.vector.tensor_copy(dst, ps_o[:])
                                else:
                                    nc.scalar.copy(dst, ps_o[:])
                        nc.sync.dma_start(out_p2[ip], outsb[:])

        if reps == 1:
            body()
        else:
            with tc.For_i(0, reps, 1):
                body()

    nc.compile()
    return nc


def _tile_weight(w_t: np.ndarray) -> np.ndarray:
    """[2048, 64] -> [128, 16*64] stack where tile t = cols [64t:64t+64]."""
    return np.ascontiguousarray(
        w_t.reshape(T, P, R).transpose(1, 0, 2).reshape(P, T * R)
    ).astype(np.float16)


def build_in_maps(hs_all: np.ndarray, w_seq: np.ndarray, w_hid: np.ndarray):
    ident = np.eye(P, dtype=np.float16)
    wst_rolls = {
        j: _tile_weight(np.roll(w_seq.T, -(S // 2) * j, axis=0)) for j in range(2)
    }
    wht_tiled = _tile_weight(np.ascontiguousarray(w_hid.T))
    hs_f16 = hs_all.astype(np.float16)
    in_maps = []
    for c in range(N_CORES):
        b, j = c // 2, c % 2
        hsb = hs_f16[b]
        hs_c = hsb if j == 0 else np.ascontiguousarray(np.roll(hsb, -(S // 2), axis=0))
        in_maps.append(
            {"hs": hs_c, "wst": wst_rolls[j], "wht": wht_tiled, "ident": ident}
        )
    return in_maps


_NC_CACHE: dict = {}


def kernel(**inputs) -> np.ndarray:
    hs_all = np.asarray(inputs["hidden_states"], dtype=np.float32)
    w_seq = np.asarray(inputs["W_seq"], dtype=np.float32)
    w_hid = np.asarray(inputs["W_hid"], dtype=np.float32)

    if "nc" not in _NC_CACHE:
        _NC_CACHE["nc"] = build_nc(1)
    nc = _NC_CACHE["nc"]

    in_maps = build_in_maps(hs_all, w_seq, w_hid)
    res = run_bass_kernel_spmd(nc, in_maps, core_ids=list(range(N_CORES)))

    out_full = np.empty((B, S, H), dtype=np.float32)
    for c in range(N_CORES):
        b, j = c // 2, c % 2
        out_full[b, j * (S // 2):(j + 1) * (S // 2), :] = res.results[c]["out"]
    return out_full
